# revision 1
# baseline (speedup 1.0000x reference)
"""TRN2 Bass kernel for nn_Attention_m_17815524344494.

Multi-head attention over [B=8, M=4, P=512, H=768], nh=12, hs=64.
Sharding: data-parallel over batch B -> one batch element per NeuronCore (8 cores).

Per-core dataflow (T = M*P = 2048 tokens; all matmul operands fp16 —
same 11-bit mantissa as float32r but FWL-eligible so LDWEIGHTS hides;
accumulation is always fp32 in PSUM):
  1. xT [768,2048] (pre-transposed on host) DMA'd feature-major per modality
  2. qT = Wq^T xT, kT = Wk^T xT (feature-major), v = x Wv (token-major,
     augmented with a ones column per head for free softmax sums)
  3. per (modality, head): scoresT = kT^T q (keys on partitions),
     eT = exp(scoresT/8) via ScalarE, ctxT_unnorm/sums = v_aug^T eT,
     1/sums via reciprocal_approx_fast, partition-broadcast through a
     DRAM bounce DMA, normalize in place on VectorE
  4. out = ctxT^T Wo (token-major), DMA to DRAM

Biases are zeros per the problem spec; a numpy fallback handles the
(never exercised) nonzero-bias case.
"""

from contextlib import ExitStack

import numpy as np

import concourse.mybir as mybir
from concourse import bacc, bass_utils
from concourse.tile import TileContext

F32 = mybir.dt.float32
F32R = mybir.dt.float32r
F16 = mybir.dt.float16
AF = mybir.ActivationFunctionType
ALU = mybir.AluOpType

B, M, PM, H = 8, 4, 512, 768
NH, HS = 12, 64
T = M * PM          # 2048 tokens per core
HC = H // 128       # 6 hidden chunks
TCM = PM // 128     # 4 token chunks per modality


def _emit(tc, ctx):
    nc = tc.nc

    x_ap = nc.dram_tensor("x", [H, T], F32, kind="ExternalInput").ap()
    wq_ap = nc.dram_tensor("wq", [H, H], F32, kind="ExternalInput").ap()
    wk_ap = nc.dram_tensor("wk", [H, H], F32, kind="ExternalInput").ap()
    wv_ap = nc.dram_tensor("wv", [H, H], F32, kind="ExternalInput").ap()
    wo_ap = nc.dram_tensor("wo", [H, H], F32, kind="ExternalInput").ap()
    out_ap = nc.dram_tensor("out", [T, H], F32, kind="ExternalOutput").ap()
    srf_ap = nc.dram_tensor("srf", [M * NH, 512], F32, kind="Internal").ap()

    const = ctx.enter_context(tc.tile_pool(name="const", bufs=1))

    # f32r tiles can't be written by memset/affine_select directly (no
    # f32r rounding on those ISA paths); stage in f32 and copy via DVE.
    onescol = const.tile([128, NH * TCM], F16)
    with tc.tile_pool(name="stage", bufs=1) as stage:
        ones_stage = stage.tile([128, 64], F32)
        nc.gpsimd.memset(ones_stage[:], 1.0)
        nc.vector.tensor_copy(onescol[:], ones_stage[:, :NH * TCM])

    wpool = ctx.enter_context(tc.tile_pool(name="w", bufs=1))
    xtp = ctx.enter_context(tc.tile_pool(name="xt", bufs=2))
    qpool = ctx.enter_context(tc.tile_pool(name="q", bufs=2))
    kpool = ctx.enter_context(tc.tile_pool(name="k", bufs=2))
    vpool = ctx.enter_context(tc.tile_pool(name="v", bufs=2))
    epool = ctx.enter_context(tc.tile_pool(name="e", bufs=8))
    stpool = ctx.enter_context(tc.tile_pool(name="st", bufs=2))
    bcpool = ctx.enter_context(tc.tile_pool(name="bc", bufs=5))
    cpool = ctx.enter_context(tc.tile_pool(name="ctx", bufs=1))
    opool = ctx.enter_context(tc.tile_pool(name="o", bufs=2))
    ps_big = ctx.enter_context(tc.tile_pool(name="ps_big", bufs=2, space="PSUM"))
    ps_sc = ctx.enter_context(tc.tile_pool(name="ps_sc", bufs=4, space="PSUM"))
    ps_c = ctx.enter_context(tc.tile_pool(name="ps_c", bufs=2, space="PSUM"))

    w_tiles = {}

    def load_weights():
        for name, ap in (("wk", wk_ap), ("wv", wv_ap), ("wo", wo_ap)):
            t = wpool.tile([128, HC, H], F16, tag=name)
            src = ap.rearrange("(kc p) j -> p kc j", p=128)
            for kc in range(HC):
                nc.gpsimd.dma_start(t[:, kc, :], src[:, kc, :])
            w_tiles[name] = t

    mod = {}

    def emit_load_x(m):
        xt = xtp.tile([128, HC, PM], F16, tag="xt")
        if m == 0:
            # Interleave x and Wq chunk DMAs so the first projection group's
            # operands land as early as possible, then stream the rest.
            wq = wpool.tile([128, HC, H], F16, tag="wq", name="wq")
            w_tiles["wq"] = wq
            wq_src = wq_ap.rearrange("(kc p) j -> p kc j", p=128)
            for hc in range(HC):
                nc.gpsimd.dma_start(
                    xt[:, hc, :],
                    x_ap.rearrange("(hc p) t -> p hc t", p=128)[:, hc, :PM],
                )
                nc.gpsimd.dma_start(wq[:, hc, :], wq_src[:, hc, :])
            mod[m] = {"xt": xt}
            load_weights()
            return
        for hc in range(HC):
            nc.gpsimd.dma_start(
                xt[:, hc, :],
                x_ap.rearrange("(hc p) t -> p hc t", p=128)[:, hc, m * PM:(m + 1) * PM],
            )
        mod[m] = {"xt": xt}

    def proj_qk_group(m, which, jc):
        st = mod[m]
        key = "qt" if which == "q" else "kt"
        if key not in st:
            pool = qpool if which == "q" else kpool
            st[key] = pool.tile([128, HC, PM], F16, tag=which, name=f"{which}t")
        w = w_tiles["wq" if which == "q" else "wk"]
        ps = ps_big.tile([128, 512], F32, tag="ps_big")
        for kc in range(HC):
            nc.tensor.matmul(
                ps[:],
                w[:, kc, jc * 128:(jc + 1) * 128],
                st["xt"][:, kc, :],
                start=(kc == 0),
                stop=(kc == HC - 1),
            )
        if jc % 2 == 0:
            nc.vector.tensor_copy(st[key][:, jc, :], ps[:])
        else:
            nc.scalar.activation(st[key][:, jc, :], ps[:], AF.Copy)

    def proj_v_group(m, ti, nn):
        st = mod[m]
        if "vt" not in st:
            st["vt"] = vpool.tile([128, TCM, NH, HS + 1], F16, tag="v", name="vt")
            nc.vector.tensor_copy(
                st["vt"][:, :, :, HS],
                onescol[:].rearrange("p (t h) -> p t h", t=TCM),
            )
        ps = ps_big.tile([128, 512], F32, tag="ps_big")
        for kc in range(HC):
            nc.tensor.matmul(
                ps[:, :384],
                st["xt"][:, kc, ti * 128:(ti + 1) * 128],
                w_tiles["wv"][:, kc, nn * 384:(nn + 1) * 384],
                start=(kc == 0),
                stop=(kc == HC - 1),
            )
        nc.scalar.activation(
            st["vt"][:, ti, nn * 6:(nn + 1) * 6, :HS],
            ps[:, :384].rearrange("p (h c) -> p h c", c=HS),
            AF.Copy,
        )

    def phase_ab_fillers(m):
        # v groups are interleaved early: their ScalarE evacuations queue
        # behind exp ops, so spreading them across the attention phase beats
        # a burst at the modality boundary.
        yield lambda: emit_load_x(m)
        order = []
        for jc in range(HC):
            order.append(("q", jc))
        for jc in range(HC):
            order.append(("k", jc))
        vlist = [(ti, nn) for ti in range(TCM) for nn in range(2)]
        merged = []
        for i, qk in enumerate(order):
            merged.append(qk)
            if i % 3 == 1 and vlist:
                merged.append(("v", vlist.pop(0)))
        merged.extend(("v", v) for v in vlist)
        for item in merged:
            if item[0] == "v":
                ti, nn = item[1]
                yield lambda ti=ti, nn=nn: proj_v_group(m, ti, nn)
            else:
                which, jc = item
                yield lambda which=which, jc=jc: proj_qk_group(m, which, jc)

    def attention(m, fillers):
        # Per (modality, head): scoresT on PE, exp on ScalarE, PV (with the
        # v_aug ones column producing softmax sums in psum row 64).
        # Normalization is batched (reciprocal_approx_fast, DMA partition
        # broadcast via a DRAM bounce, in-place scale) so the PE never waits
        # on the recip chain. Between each head's scores and PV we weave one
        # projection group of the NEXT modality -- independent PE work that
        # fills the exp wait.
        st = mod[m]
        qt, kt, vt = st["qt"], st["kt"], st["vt"]
        ctxt = cpool.tile([128, HC, PM], F16, tag="ctx")
        st["ctxt"] = ctxt
        bcs = []

        def normalize(heads):
            for h in heads:
                hc, hr = h // 2, (h % 2) * 64
                nc.vector.tensor_tensor(
                    ctxt[hr:hr + 64, hc, :], ctxt[hr:hr + 64, hc, :],
                    bcs[h][hr:hr + 64, :], ALU.mult,
                )

        for h in range(NH):
            hc, hr = h // 2, (h % 2) * 64
            qh = qt[hr:hr + 64, hc, :]
            ets = []
            for jc in range(TCM):
                pssc = ps_sc.tile([128, 512], F32, tag="ps_sc")
                nc.tensor.matmul(
                    pssc[:],
                    kt[hr:hr + 64, hc, jc * 128:(jc + 1) * 128],
                    qh,
                    start=True,
                    stop=True,
                )
                et = epool.tile([128, 512], F16, tag="e")
                nc.scalar.activation(et[:], pssc[:], AF.Exp, scale=0.125)
                ets.append(et)
            for f in fillers[:1]:
                f()
            del fillers[:1]
            psc = ps_c.tile([HS + 1, 512], F32, tag="ps_c")
            for jc in range(TCM):
                nc.tensor.matmul(
                    psc[:],
                    vt[:, jc, h, :],
                    ets[jc][:],
                    start=(jc == 0),
                    stop=(jc == TCM - 1),
                )
            nc.vector.tensor_copy(ctxt[hr:hr + 64, hc, :], psc[:HS, :])
            stmp = stpool.tile([1, 512], F32, tag="stmp")
            nc.vector.tensor_copy(stmp[:], psc[HS:HS + 1, :])
            rf = stpool.tile([1, 512], F32, tag="rf")
            nc.vector.reciprocal_approx_fast(out=rf[:], in_=stmp[:])
            row = srf_ap[m * NH + h:m * NH + h + 1, :]
            nc.sync.dma_start(row, rf[0:1, :])
            bc = bcpool.tile([128, 512], F32, tag="bc")
            nc.sync.dma_start(bc[:], row.to_broadcast((128, 512)))
            bcs.append(bc)
            if len(bcs) in (4, 8):
                normalize(range(len(bcs) - 4, len(bcs)))
        for f in fillers:
            f()
        del fillers[:]
        normalize(range(8, NH))


    def out_proj(m):
        ctxt = mod[m]["ctxt"]
        for ti in range(TCM):
            osb = opool.tile([128, H], F32, tag="o")
            for nn in range(2):
                ps = ps_big.tile([128, 512], F32, tag="ps_big")
                for cc in range(HC):
                    nc.tensor.matmul(
                        ps[:, :384],
                        ctxt[:, cc, ti * 128:(ti + 1) * 128],
                        w_tiles["wo"][:, cc, nn * 384:(nn + 1) * 384],
                        start=(cc == 0),
                        stop=(cc == HC - 1),
                    )
                nc.scalar.activation(osb[:, nn * 384:(nn + 1) * 384], ps[:, :384], AF.Copy)
            row0 = (m * TCM + ti) * 128
            nc.sync.dma_start(out_ap[row0:row0 + 128, :], osb[:])

    # Modality 0 bootstrap: kc-outer paired projection consumes x/W DMA
    # chunks as they arrive instead of waiting for whole tensors.
    emit_load_x(0)
    for which in ("q", "k"):
        st0 = mod[0]
        key = "qt" if which == "q" else "kt"
        st0[key] = (qpool if which == "q" else kpool).tile(
            [128, HC, PM], F16, tag=which, name=f"{which}t0")
        w = w_tiles["wq" if which == "q" else "wk"]
        for jcp in range(3):
            psA = ps_big.tile([128, 512], F32, tag="ps_big")
            psB = ps_big.tile([128, 512], F32, tag="ps_big")
            for kc in range(HC):
                nc.tensor.matmul(
                    psA[:], w[:, kc, (2 * jcp) * 128:(2 * jcp + 1) * 128],
                    st0["xt"][:, kc, :], start=(kc == 0), stop=(kc == HC - 1))
                nc.tensor.matmul(
                    psB[:], w[:, kc, (2 * jcp + 1) * 128:(2 * jcp + 2) * 128],
                    st0["xt"][:, kc, :], start=(kc == 0), stop=(kc == HC - 1))
            nc.vector.tensor_copy(st0[key][:, 2 * jcp, :], psA[:])
            nc.vector.tensor_copy(st0[key][:, 2 * jcp + 1, :], psB[:])
    for ti in range(TCM):
        for nn in range(2):
            proj_v_group(0, ti, nn)
    for m in range(M):
        fillers = list(phase_ab_fillers(m + 1)) if m + 1 < M else []
        attention(m, fillers)
        out_proj(m)

_NC_CACHE = {}


def build_nc():
    if "nc" not in _NC_CACHE:
        nc = bacc.Bacc("TRN2", target_bir_lowering=False, debug=False, num_devices=B)
        with TileContext(nc) as tc:
            with ExitStack() as stack:
                _emit(tc, stack)
        nc.compile()
        _NC_CACHE["nc"] = nc
    return _NC_CACHE["nc"]


def _numpy_fallback(x, Wq, bq, Wk, bk, Wv, bv, Wo, bo):
    Bb, Mm, Pp, Hh = x.shape
    xx = x.reshape(-1, Hh)
    q = (xx @ Wq + bq).reshape(Bb, Mm, Pp, NH, HS).transpose(0, 1, 3, 2, 4)
    k = (xx @ Wk + bk).reshape(Bb, Mm, Pp, NH, HS).transpose(0, 1, 3, 2, 4)
    v = (xx @ Wv + bv).reshape(Bb, Mm, Pp, NH, HS).transpose(0, 1, 3, 2, 4)
    s = np.einsum("bmnqh,bmnkh->bmnqk", q, k) / np.sqrt(HS)
    s = s - s.max(axis=-1, keepdims=True)
    e = np.exp(s)
    p = e / e.sum(axis=-1, keepdims=True)
    ctx = np.einsum("bmnqk,bmnkh->bmnqh", p, v)
    ctx = ctx.transpose(0, 1, 3, 2, 4).reshape(Bb, Mm, Pp, Hh)
    return (ctx @ Wo + bo).astype(np.float32)


def kernel(hidden_states, Wq, bq, Wk, bk, Wv, bv, Wo, bo):
    hs = np.ascontiguousarray(np.asarray(hidden_states, dtype=np.float32))
    ws = {n: np.ascontiguousarray(np.asarray(w, dtype=np.float32))
          for n, w in (("wq", Wq), ("wk", Wk), ("wv", Wv), ("wo", Wo))}
    biases = [np.asarray(b, dtype=np.float32) for b in (bq, bk, bv, bo)]
    if any(np.any(b) for b in biases):
        return _numpy_fallback(hs, ws["wq"], biases[0], ws["wk"], biases[1],
                               ws["wv"], biases[2], ws["wo"], biases[3])

    in_maps = [
        {"x": np.ascontiguousarray(hs[b].reshape(T, H).T), **ws}
        for b in range(B)
    ]
    # The device occasionally comes up wedged from a previous process
    # (NRT_EXEC_UNIT_UNRECOVERABLE); retry, then degrade to the (correct
    # but slow) numpy path rather than crash.
    last_exc = None
    for _ in range(3):
        try:
            nc = build_nc()
            res = bass_utils.run_bass_kernel_spmd(
                nc, in_maps, core_ids=list(range(B)))
            out = np.stack(
                [res.results[b]["out"].reshape(M, PM, H) for b in range(B)])
            return out.astype(np.float32)
        except Exception as e:  # noqa: BLE001
            last_exc = e
            import time
            time.sleep(2)
    import warnings
    warnings.warn(f"TRN execution failed ({last_exc!r}); numpy fallback")
    return _numpy_fallback(hs, ws["wq"], biases[0], ws["wk"], biases[1],
                           ws["wv"], biases[2], ws["wo"], biases[3])



# revision 4
# speedup vs baseline: 1.0574x; 1.0574x over previous
"""TRN2 Bass kernel for nn_Attention_m_17815524344494.

Multi-head attention over [B=8, M=4, P=512, H=768], nh=12, hs=64.
Sharding: data-parallel over batch B -> one batch element per NeuronCore (8 cores).

Per-core dataflow (T = M*P = 2048 tokens; all matmul operands fp16 —
FWL-eligible so LDWEIGHTS hides; accumulation is fp32 in PSUM):
  1. xT [768,2048] fp16 (pre-transposed+cast on host) DMA'd feature-major
     per modality on the gpsimd queue; weights fp16 split across the two
     HW-DGE queues (wq/wo on sync, wk/wv on scalar) so the bootstrap is
     not serialized on one software-DGE queue.
  2. qT = Wq^T xT, kT = Wk^T xT (feature-major), v = x Wv (token-major,
     augmented with a ones column per head for free softmax sums)
  3. per (modality, head): scoresT = kT^T q (keys on partitions),
     eT = exp(scoresT/8) via ScalarE, ctxT_unnorm/sums = v_aug^T eT,
     1/sums via reciprocal_approx_fast straight out of PSUM, partition-
     broadcast to 64 rows through a DRAM bounce DMA, normalize in place
     on VectorE two heads behind the producer (keeps the DVE queue from
     backing up at modality boundaries)
  4. out = ctxT^T Wo (token-major), f16 out DMA, host casts back to f32

PE idle is filled by weaving independent work into each attention
phase: modality m's attention interleaves modality m+1's projections,
and the last modality's attention interleaves modality m-1's output
projection (which is why the ctx pool is double-buffered).

Biases are zeros per the problem spec; a numpy fallback handles the
(never exercised) nonzero-bias case.
"""

from contextlib import ExitStack

import numpy as np

import concourse.mybir as mybir
from concourse import bacc, bass_utils
from concourse.tile import TileContext

F32 = mybir.dt.float32
F16 = mybir.dt.float16
AF = mybir.ActivationFunctionType
ALU = mybir.AluOpType

B, M, PM, H = 8, 4, 512, 768
NH, HS = 12, 64
T = M * PM          # 2048 tokens per core
HC = H // 128       # 6 hidden chunks
TCM = PM // 128     # 4 token chunks per modality


def _emit(tc, ctx):
    nc = tc.nc

    x_ap = nc.dram_tensor("x", [H, T], F16, kind="ExternalInput").ap()
    wq_ap = nc.dram_tensor("wq", [H, H], F16, kind="ExternalInput").ap()
    wk_ap = nc.dram_tensor("wk", [H, H], F16, kind="ExternalInput").ap()
    wv_ap = nc.dram_tensor("wv", [H, H], F16, kind="ExternalInput").ap()
    wo_ap = nc.dram_tensor("wo", [H, H], F16, kind="ExternalInput").ap()
    out_ap = nc.dram_tensor("out", [T, H], F16, kind="ExternalOutput").ap()
    srf_ap = nc.dram_tensor("srf", [M * NH, 512], F32, kind="Internal").ap()

    const = ctx.enter_context(tc.tile_pool(name="const", bufs=1))

    onescol = const.tile([128, NH * TCM], F16)
    with tc.tile_pool(name="stage", bufs=1) as stage:
        ones_stage = stage.tile([128, 64], F32)
        nc.gpsimd.memset(ones_stage[:], 1.0)
        nc.vector.tensor_copy(onescol[:], ones_stage[:, :NH * TCM])

    wpool = ctx.enter_context(tc.tile_pool(name="w", bufs=1))
    xtp = ctx.enter_context(tc.tile_pool(name="xt", bufs=2))
    qpool = ctx.enter_context(tc.tile_pool(name="q", bufs=2))
    kpool = ctx.enter_context(tc.tile_pool(name="k", bufs=2))
    vpool = ctx.enter_context(tc.tile_pool(name="v", bufs=2))
    epool = ctx.enter_context(tc.tile_pool(name="e", bufs=8))
    stpool = ctx.enter_context(tc.tile_pool(name="st", bufs=2))
    bcpool = ctx.enter_context(tc.tile_pool(name="bc", bufs=4))
    cpool = ctx.enter_context(tc.tile_pool(name="ctx", bufs=2))
    opool = ctx.enter_context(tc.tile_pool(name="o", bufs=2))
    ps_big = ctx.enter_context(tc.tile_pool(name="ps_big", bufs=2, space="PSUM"))
    ps_sc = ctx.enter_context(tc.tile_pool(name="ps_sc", bufs=4, space="PSUM"))
    ps_c = ctx.enter_context(tc.tile_pool(name="ps_c", bufs=2, space="PSUM"))

    w_tiles = {}
    mod = {}

    def emit_load_x(m):
        xt = xtp.tile([128, HC, PM], F16, tag="xt")
        xsrc = x_ap.rearrange("(hc p) t -> p hc t", p=128)
        for hc in range(HC):
            nc.gpsimd.dma_start(xt[:, hc, :], xsrc[:, hc, m * PM:(m + 1) * PM])
        mod[m] = {"xt": xt}
        if m == 0:
            # Weights ride the two HW-DGE queues in parallel with x on
            # gpsimd; first-needed tensors lead each queue.
            for name, ap, eng in (("wq", wq_ap, nc.sync), ("wk", wk_ap, nc.scalar),
                                  ("wv", wv_ap, nc.scalar), ("wo", wo_ap, nc.sync)):
                t = wpool.tile([128, HC, H], F16, tag=name, name=name)
                src = ap.rearrange("(kc p) j -> p kc j", p=128)
                for kc in range(HC):
                    eng.dma_start(t[:, kc, :], src[:, kc, :])
                w_tiles[name] = t

    def proj_qk_group(m, which, jc):
        st = mod[m]
        key = "qt" if which == "q" else "kt"
        if key not in st:
            pool = qpool if which == "q" else kpool
            st[key] = pool.tile([128, HC, PM], F16, tag=which, name=f"{which}t")
        w = w_tiles["wq" if which == "q" else "wk"]
        ps = ps_big.tile([128, 512], F32, tag="ps_big")
        for kc in range(HC):
            nc.tensor.matmul(
                ps[:],
                w[:, kc, jc * 128:(jc + 1) * 128],
                st["xt"][:, kc, :],
                start=(kc == 0),
                stop=(kc == HC - 1),
            )
        if jc % 2 == 0:
            nc.vector.tensor_copy(st[key][:, jc, :], ps[:])
        else:
            nc.scalar.activation(st[key][:, jc, :], ps[:], AF.Copy)

    def proj_v_group(m, ti, nn):
        st = mod[m]
        if "vt" not in st:
            st["vt"] = vpool.tile([128, TCM, NH, HS + 1], F16, tag="v", name="vt")
            nc.vector.tensor_copy(
                st["vt"][:, :, :, HS],
                onescol[:].rearrange("p (t h) -> p t h", t=TCM),
            )
        ps = ps_big.tile([128, 512], F32, tag="ps_big")
        for kc in range(HC):
            nc.tensor.matmul(
                ps[:, :384],
                st["xt"][:, kc, ti * 128:(ti + 1) * 128],
                w_tiles["wv"][:, kc, nn * 384:(nn + 1) * 384],
                start=(kc == 0),
                stop=(kc == HC - 1),
            )
        nc.scalar.activation(
            st["vt"][:, ti, nn * 6:(nn + 1) * 6, :HS],
            ps[:, :384].rearrange("p (h c) -> p h c", c=HS),
            AF.Copy,
        )

    def phase_ab_fillers(m):
        # v groups are interleaved early: their ScalarE evacuations queue
        # behind exp ops, so spreading them across the attention phase beats
        # a burst at the modality boundary.
        yield lambda: emit_load_x(m)
        order = []
        for jc in range(HC):
            order.append(("q", jc))
        for jc in range(HC):
            order.append(("k", jc))
        vlist = [(ti, nn) for ti in range(TCM) for nn in range(2)]
        merged = []
        for i, qk in enumerate(order):
            merged.append(qk)
            if i % 3 == 1 and vlist:
                merged.append(("v", vlist.pop(0)))
        merged.extend(("v", v) for v in vlist)
        for item in merged:
            if item[0] == "v":
                ti, nn = item[1]
                yield lambda ti=ti, nn=nn: proj_v_group(m, ti, nn)
            else:
                which, jc = item
                yield lambda which=which, jc=jc: proj_qk_group(m, which, jc)

    def out_proj_piece(m, ti, nn, osbs):
        ctxt = mod[m]["ctxt"]
        if nn == 0:
            osbs[ti] = opool.tile([128, H], F16, tag="o", name="osb")
        osb = osbs[ti]
        ps = ps_big.tile([128, 512], F32, tag="ps_big")
        for cc in range(HC):
            nc.tensor.matmul(
                ps[:, :384],
                ctxt[:, cc, ti * 128:(ti + 1) * 128],
                w_tiles["wo"][:, cc, nn * 384:(nn + 1) * 384],
                start=(cc == 0),
                stop=(cc == HC - 1),
            )
        nc.scalar.activation(osb[:, nn * 384:(nn + 1) * 384], ps[:, :384], AF.Copy)
        if nn == 1:
            row0 = (m * TCM + ti) * 128
            nc.sync.dma_start(out_ap[row0:row0 + 128, :], osb[:])

    def out_proj_fillers(m):
        osbs = {}
        return [
            (lambda ti=ti, nn=nn: out_proj_piece(m, ti, nn, osbs))
            for ti in range(TCM) for nn in range(2)
        ]

    def out_proj(m):
        for f in out_proj_fillers(m):
            f()

    def attention(m, fillers):
        # Per (modality, head): scoresT on PE, exp on ScalarE, PV (with the
        # v_aug ones column producing softmax sums in psum row 64).
        # 1/sums comes straight off PSUM via reciprocal_approx_fast, is
        # partition-broadcast through a DRAM bounce DMA into the head's own
        # 64 rows, and the in-place normalize trails the producer by two
        # heads so the (in-order) DVE queue never gates the PE. Between each
        # head's scores and PV one filler runs -- independent PE work that
        # fills the exp wait.
        st = mod[m]
        qt, kt, vt = st["qt"], st["kt"], st["vt"]
        ctxt = cpool.tile([128, HC, PM], F16, tag="ctx")
        st["ctxt"] = ctxt
        pending = []

        def normalize_one():
            h, bc = pending.pop(0)
            hc, hr = h // 2, (h % 2) * 64
            nc.vector.tensor_tensor(
                ctxt[hr:hr + 64, hc, :], ctxt[hr:hr + 64, hc, :],
                bc[hr:hr + 64, :], ALU.mult,
            )

        for h in range(NH):
            hc, hr = h // 2, (h % 2) * 64
            qh = qt[hr:hr + 64, hc, :]
            ets = []
            for jc in range(TCM):
                pssc = ps_sc.tile([128, 512], F32, tag="ps_sc")
                nc.tensor.matmul(
                    pssc[:],
                    kt[hr:hr + 64, hc, jc * 128:(jc + 1) * 128],
                    qh,
                    start=True,
                    stop=True,
                )
                et = epool.tile([128, 512], F16, tag="e")
                nc.scalar.activation(et[:], pssc[:], AF.Exp, scale=0.125)
                ets.append(et)
            if fillers:
                fillers.pop(0)()
            psc = ps_c.tile([HS + 1, 512], F32, tag="ps_c")
            for jc in range(TCM):
                nc.tensor.matmul(
                    psc[:],
                    vt[:, jc, h, :],
                    ets[jc][:],
                    start=(jc == 0),
                    stop=(jc == TCM - 1),
                )
            nc.vector.tensor_copy(ctxt[hr:hr + 64, hc, :], psc[:HS, :])
            stmp = stpool.tile([1, 512], F32, tag="stmp")
            nc.vector.tensor_copy(stmp[:], psc[HS:HS + 1, :])
            rf = stpool.tile([1, 512], F32, tag="rf")
            nc.vector.reciprocal_approx_fast(out=rf[:], in_=stmp[:])
            row = srf_ap[m * NH + h:m * NH + h + 1, :]
            nc.sync.dma_start(row, rf[0:1, :])
            bc = bcpool.tile([128, 512], F32, tag="bc")
            nc.sync.dma_start(bc[hr:hr + 64, :], row.to_broadcast((64, 512)))
            pending.append((h, bc))
            while len(pending) > 2:
                normalize_one()
        for f in fillers:
            f()
        del fillers[:]
        while pending:
            normalize_one()

    # Modality 0 bootstrap: kc-outer paired projection consumes x/W DMA
    # chunks as they arrive instead of waiting for whole tensors.
    emit_load_x(0)
    for which in ("q", "k"):
        st0 = mod[0]
        key = "qt" if which == "q" else "kt"
        st0[key] = (qpool if which == "q" else kpool).tile(
            [128, HC, PM], F16, tag=which, name=f"{which}t0")
        w = w_tiles["wq" if which == "q" else "wk"]
        for jcp in range(3):
            psA = ps_big.tile([128, 512], F32, tag="ps_big")
            psB = ps_big.tile([128, 512], F32, tag="ps_big")
            for kc in range(HC):
                nc.tensor.matmul(
                    psA[:], w[:, kc, (2 * jcp) * 128:(2 * jcp + 1) * 128],
                    st0["xt"][:, kc, :], start=(kc == 0), stop=(kc == HC - 1))
                nc.tensor.matmul(
                    psB[:], w[:, kc, (2 * jcp + 1) * 128:(2 * jcp + 2) * 128],
                    st0["xt"][:, kc, :], start=(kc == 0), stop=(kc == HC - 1))
            nc.vector.tensor_copy(st0[key][:, 2 * jcp, :], psA[:])
            nc.vector.tensor_copy(st0[key][:, 2 * jcp + 1, :], psB[:])
    for ti in range(TCM):
        for nn in range(2):
            proj_v_group(0, ti, nn)

    attention(0, list(phase_ab_fillers(1)))
    out_proj(0)
    attention(1, list(phase_ab_fillers(2)))
    out_proj(1)
    attention(2, list(phase_ab_fillers(3)))
    attention(3, out_proj_fillers(2))
    out_proj(3)


_NC_CACHE = {}


def build_nc():
    if "nc" not in _NC_CACHE:
        nc = bacc.Bacc("TRN2", target_bir_lowering=False, debug=False, num_devices=B)
        with TileContext(nc) as tc:
            with ExitStack() as stack:
                _emit(tc, stack)
        nc.compile()
        _NC_CACHE["nc"] = nc
    return _NC_CACHE["nc"]


def prep_in_maps(hidden_states, Wq, Wk, Wv, Wo):
    hs = np.asarray(hidden_states, dtype=np.float32)
    ws = {n: np.ascontiguousarray(np.asarray(w, dtype=np.float32)).astype(np.float16)
          for n, w in (("wq", Wq), ("wk", Wk), ("wv", Wv), ("wo", Wo))}
    return [
        {"x": np.ascontiguousarray(hs[b].reshape(T, H).T).astype(np.float16), **ws}
        for b in range(B)
    ]


def _numpy_fallback(x, Wq, bq, Wk, bk, Wv, bv, Wo, bo):
    Bb, Mm, Pp, Hh = x.shape
    xx = x.reshape(-1, Hh)
    q = (xx @ Wq + bq).reshape(Bb, Mm, Pp, NH, HS).transpose(0, 1, 3, 2, 4)
    k = (xx @ Wk + bk).reshape(Bb, Mm, Pp, NH, HS).transpose(0, 1, 3, 2, 4)
    v = (xx @ Wv + bv).reshape(Bb, Mm, Pp, NH, HS).transpose(0, 1, 3, 2, 4)
    s = np.einsum("bmnqh,bmnkh->bmnqk", q, k) / np.sqrt(HS)
    s = s - s.max(axis=-1, keepdims=True)
    e = np.exp(s)
    p = e / e.sum(axis=-1, keepdims=True)
    ctx = np.einsum("bmnqk,bmnkh->bmnqh", p, v)
    ctx = ctx.transpose(0, 1, 3, 2, 4).reshape(Bb, Mm, Pp, Hh)
    return (ctx @ Wo + bo).astype(np.float32)


def kernel(hidden_states, Wq, bq, Wk, bk, Wv, bv, Wo, bo):
    hs = np.asarray(hidden_states, dtype=np.float32)
    biases = [np.asarray(b, dtype=np.float32) for b in (bq, bk, bv, bo)]
    if any(np.any(b) for b in biases):
        return _numpy_fallback(hs, np.asarray(Wq, dtype=np.float32), biases[0],
                               np.asarray(Wk, dtype=np.float32), biases[1],
                               np.asarray(Wv, dtype=np.float32), biases[2],
                               np.asarray(Wo, dtype=np.float32), biases[3])

    in_maps = prep_in_maps(hs, Wq, Wk, Wv, Wo)
    # The device occasionally comes up wedged from a previous process
    # (NRT_EXEC_UNIT_UNRECOVERABLE); retry, then degrade to the (correct
    # but slow) numpy path rather than crash.
    last_exc = None
    for _ in range(3):
        try:
            nc = build_nc()
            res = bass_utils.run_bass_kernel_spmd(
                nc, in_maps, core_ids=list(range(B)))
            out = np.stack(
                [res.results[b]["out"].reshape(M, PM, H).astype(np.float32)
                 for b in range(B)])
            return out
        except Exception as e:  # noqa: BLE001
            last_exc = e
            import time
            time.sleep(2)
    import warnings
    warnings.warn(f"TRN execution failed ({last_exc!r}); numpy fallback")
    return _numpy_fallback(hs, np.asarray(Wq, dtype=np.float32), biases[0],
                           np.asarray(Wk, dtype=np.float32), biases[1],
                           np.asarray(Wv, dtype=np.float32), biases[2],
                           np.asarray(Wo, dtype=np.float32), biases[3])


# revision 12
# speedup vs baseline: 1.0801x; 1.0215x over previous
"""TRN2 Bass kernel for nn_Attention_m_17815524344494.

Multi-head attention over [B=8, M=4, P=512, H=768], nh=12, hs=64.
Sharding: data-parallel over batch B -> one batch element per NeuronCore (8 cores).

Per-core dataflow (T = M*P = 2048 tokens; all matmul operands fp16 —
FWL-eligible so LDWEIGHTS hides; accumulation is fp32 in PSUM):
  1. xT [768,2048] fp16 (pre-transposed+cast on host) DMA'd feature-major
     per modality on the gpsimd queue; weights fp16 split across the two
     HW-DGE queues (wq/wo on sync, wk/wv on scalar) so the bootstrap is
     not serialized on one software-DGE queue.
  2. qT = Wq^T xT, kT = Wk^T xT (feature-major), v = x Wv (token-major,
     augmented with a ones column per head for free softmax sums)
  3. per (modality, head): scoresT = kT^T q (keys on partitions),
     eT = exp(scoresT/8) via ScalarE, ctxT_unnorm/sums = v_aug^T eT,
     1/sums via reciprocal_approx_fast straight out of PSUM, partition-
     broadcast to 64 rows through a DRAM bounce DMA, normalize in place
     on VectorE two heads behind the producer (keeps the DVE queue from
     backing up at modality boundaries)
  4. out = ctxT^T Wo (token-major), f16 out DMA, host casts back to f32

PE idle is filled by weaving independent work into each attention
phase: modality m's attention interleaves modality m+1's projections,
and the last modality's attention interleaves modality m-1's output
projection (which is why the ctx pool is double-buffered).

Biases are zeros per the problem spec; a numpy fallback handles the
(never exercised) nonzero-bias case.
"""

from contextlib import ExitStack

import numpy as np

import concourse.mybir as mybir
from concourse import bacc, bass_utils
from concourse.tile import TileContext

F32 = mybir.dt.float32
F16 = mybir.dt.float16
AF = mybir.ActivationFunctionType
ALU = mybir.AluOpType

B, M, PM, H = 8, 4, 512, 768
NH, HS = 12, 64
T = M * PM          # 2048 tokens per core
HC = H // 128       # 6 hidden chunks
TCM = PM // 128     # 4 token chunks per modality


def _emit(tc, ctx):
    nc = tc.nc

    x_ap = nc.dram_tensor("x", [H, T], F16, kind="ExternalInput").ap()
    wq_ap = nc.dram_tensor("wq", [H, H], F16, kind="ExternalInput").ap()
    wk_ap = nc.dram_tensor("wk", [H, H], F16, kind="ExternalInput").ap()
    wv_ap = nc.dram_tensor("wv", [H, H], F16, kind="ExternalInput").ap()
    wo_ap = nc.dram_tensor("wo", [H, H], F16, kind="ExternalInput").ap()
    # Output stays feature-major [H, T]; the host transposes. This lets
    # out-proj run with Wo chunks stationary and ctxT moving (512-col
    # matmuls that hide LDWEIGHTS, and cc-accumulation that can start
    # before the last heads are normalized).
    out_ap = nc.dram_tensor("out", [H, T], F16, kind="ExternalOutput").ap()
    srf_ap = nc.dram_tensor("srf", [M * NH, 512], F32, kind="Internal").ap()

    const = ctx.enter_context(tc.tile_pool(name="const", bufs=1))

    onescol = const.tile([128, NH * TCM], F16)
    with tc.tile_pool(name="stage", bufs=1) as stage:
        ones_stage = stage.tile([128, 64], F32)
        nc.gpsimd.memset(ones_stage[:], 1.0)
        nc.vector.tensor_copy(onescol[:], ones_stage[:, :NH * TCM])

    wpool = ctx.enter_context(tc.tile_pool(name="w", bufs=1))
    xtp = ctx.enter_context(tc.tile_pool(name="xt", bufs=2))
    qpool = ctx.enter_context(tc.tile_pool(name="q", bufs=2))
    kpool = ctx.enter_context(tc.tile_pool(name="k", bufs=2))
    vpool = ctx.enter_context(tc.tile_pool(name="v", bufs=2))
    epool = ctx.enter_context(tc.tile_pool(name="e", bufs=8))
    stpool = ctx.enter_context(tc.tile_pool(name="st", bufs=2))
    bcpool = ctx.enter_context(tc.tile_pool(name="bc", bufs=4))
    cpool = ctx.enter_context(tc.tile_pool(name="ctx", bufs=2))
    opool = ctx.enter_context(tc.tile_pool(name="o", bufs=2))
    ps_big = ctx.enter_context(tc.tile_pool(name="ps_big", bufs=2, space="PSUM"))
    ps_sc = ctx.enter_context(tc.tile_pool(name="ps_sc", bufs=4, space="PSUM"))
    ps_c = ctx.enter_context(tc.tile_pool(name="ps_c", bufs=2, space="PSUM"))

    w_tiles = {}
    mod = {}

    def emit_load_x(m):
        xt = xtp.tile([128, HC, PM], F16, tag="xt")
        xsrc = x_ap.rearrange("(hc p) t -> p hc t", p=128)
        eng = nc.sync if m == 0 else nc.gpsimd
        for hc in range(HC):
            eng.dma_start(xt[:, hc, :], xsrc[:, hc, m * PM:(m + 1) * PM])
        mod[m] = {"xt": xt}
        if m == 0:
            # Startup criticality order: x+wq pace the q bootstrap, wk is
            # needed ~3us later, wv/wo later still. x and wq lead the two
            # HW-DGE queues (precise completion semaphores); wk rides the
            # leftover HW-DGE slots; wv/wo tolerate gpsimd's laggy
            # software-DGE completion visibility.
            srcs = {}
            for name, ap in (("wq", wq_ap), ("wk", wk_ap),
                             ("wv", wv_ap), ("wo", wo_ap)):
                w_tiles[name] = wpool.tile([128, HC, H], F16, tag=name, name=name)
                srcs[name] = ap.rearrange("(kc p) j -> p kc j", p=128)

            def wdma(eng, name, kc):
                eng.dma_start(w_tiles[name][:, kc, :], srcs[name][:, kc, :])

            for kc in range(HC):
                wdma(nc.scalar, "wq", kc)
            for kc in range(3):
                wdma(nc.scalar, "wk", kc)
            for kc in range(3, HC):
                wdma(nc.sync, "wk", kc)
            for kc in range(HC):
                wdma(nc.gpsimd, "wv", kc)
            for kc in range(HC):
                wdma(nc.gpsimd, "wo", kc)

    def proj_qk_group(m, which, jc):
        st = mod[m]
        key = "qt" if which == "q" else "kt"
        if key not in st:
            pool = qpool if which == "q" else kpool
            st[key] = pool.tile([128, HC, PM], F16, tag=which, name=f"{which}t")
        w = w_tiles["wq" if which == "q" else "wk"]
        ps = ps_big.tile([128, 512], F32, tag="ps_big")
        for kc in range(HC):
            nc.tensor.matmul(
                ps[:],
                w[:, kc, jc * 128:(jc + 1) * 128],
                st["xt"][:, kc, :],
                start=(kc == 0),
                stop=(kc == HC - 1),
            )
        if jc % 2 == 0:
            nc.vector.tensor_copy(st[key][:, jc, :], ps[:])
        else:
            nc.scalar.activation(st[key][:, jc, :], ps[:], AF.Copy)

    def proj_v_group(m, ti, nn):
        st = mod[m]
        if "vt" not in st:
            st["vt"] = vpool.tile([128, TCM, NH, HS + 1], F16, tag="v", name="vt")
            nc.vector.tensor_copy(
                st["vt"][:, :, :, HS],
                onescol[:].rearrange("p (t h) -> p t h", t=TCM),
            )
        ps = ps_big.tile([128, 512], F32, tag="ps_big")
        for kc in range(HC):
            nc.tensor.matmul(
                ps[:, :384],
                st["xt"][:, kc, ti * 128:(ti + 1) * 128],
                w_tiles["wv"][:, kc, nn * 384:(nn + 1) * 384],
                start=(kc == 0),
                stop=(kc == HC - 1),
            )
        nc.scalar.activation(
            st["vt"][:, ti, nn * 6:(nn + 1) * 6, :HS],
            ps[:, :384].rearrange("p (h c) -> p h c", c=HS),
            AF.Copy,
        )

    def phase_ab_fillers(m):
        # v groups are interleaved early: their ScalarE evacuations queue
        # behind exp ops, so spreading them across the attention phase beats
        # a burst at the modality boundary.
        yield lambda: emit_load_x(m)
        order = []
        for jc in range(HC):
            order.append(("q", jc))
        for jc in range(HC):
            order.append(("k", jc))
        vlist = [(ti, nn) for ti in range(TCM) for nn in range(2)]
        merged = []
        for i, qk in enumerate(order):
            merged.append(qk)
            if i % 3 == 1 and vlist:
                merged.append(("v", vlist.pop(0)))
        merged.extend(("v", v) for v in vlist)
        for item in merged:
            if item[0] == "v":
                ti, nn = item[1]
                yield lambda ti=ti, nn=nn: proj_v_group(m, ti, nn)
            else:
                which, jc = item
                yield lambda which=which, jc=jc: proj_qk_group(m, which, jc)

    out_dst = out_ap.rearrange("(oc p) t -> p oc t", p=128)

    def out_proj_piece(m, oc, osbs):
        # outT[oc*128: , m*512: ] = sum_cc Wo[cc,oc]^T ctxT[cc] -- Wo chunk
        # stationary, ctxT moving (512 cols hides LDWEIGHTS). cc runs in
        # order, so the first 4 matmuls only need heads 0..7 normalized and
        # the piece overlaps the tail of the attention normalize chain.
        ctxt = mod[m]["ctxt"]
        if oc == 0:
            osbs[m] = opool.tile([128, HC, PM], F16, tag="o", name="osb")
        osb = osbs[m]
        ps = ps_big.tile([128, 512], F32, tag="ps_big")
        for cc in range(HC):
            nc.tensor.matmul(
                ps[:],
                w_tiles["wo"][:, cc, oc * 128:(oc + 1) * 128],
                ctxt[:, cc, :],
                start=(cc == 0),
                stop=(cc == HC - 1),
            )
        nc.scalar.activation(osb[:, oc, :], ps[:], AF.Copy)
        nc.sync.dma_start(
            out_dst[:, oc, m * PM:(m + 1) * PM], osb[:, oc, :])

    def out_proj_fillers(m):
        osbs = {}
        return [
            (lambda oc=oc: out_proj_piece(m, oc, osbs))
            for oc in range(HC)
        ]

    def out_proj(m):
        for f in out_proj_fillers(m):
            f()

    def attention(m, fillers):
        # Per (modality, head): scoresT on PE, exp on ScalarE, PV (with the
        # v_aug ones column producing softmax sums in psum row 64).
        # 1/sums comes straight off PSUM via reciprocal_approx_fast, is
        # partition-broadcast through a DRAM bounce DMA into the head's own
        # 64 rows, and the in-place normalize trails the producer by two
        # heads so the (in-order) DVE queue never gates the PE. Between each
        # head's scores and PV one filler runs -- independent PE work that
        # fills the exp wait.
        st = mod[m]
        qt, kt, vt = st["qt"], st["kt"], st["vt"]
        ctxt = cpool.tile([128, HC, PM], F16, tag="ctx")
        st["ctxt"] = ctxt
        pending = []

        def normalize_one():
            h, bc = pending.pop(0)
            hc, hr = h // 2, (h % 2) * 64
            nc.vector.tensor_tensor(
                ctxt[hr:hr + 64, hc, :], ctxt[hr:hr + 64, hc, :],
                bc[hr:hr + 64, :], ALU.mult,
            )

        for h in range(NH):
            hc, hr = h // 2, (h % 2) * 64
            qh = qt[hr:hr + 64, hc, :]
            ets = []
            for jc in range(TCM):
                pssc = ps_sc.tile([128, 512], F32, tag="ps_sc")
                nc.tensor.matmul(
                    pssc[:],
                    kt[hr:hr + 64, hc, jc * 128:(jc + 1) * 128],
                    qh,
                    start=True,
                    stop=True,
                )
                et = epool.tile([128, 512], F16, tag="e")
                nc.scalar.activation(et[:], pssc[:], AF.Exp, scale=0.125)
                ets.append(et)
            if fillers:
                fillers.pop(0)()
            psc = ps_c.tile([HS + 1, 512], F32, tag="ps_c")
            for jc in range(TCM):
                nc.tensor.matmul(
                    psc[:],
                    vt[:, jc, h, :],
                    ets[jc][:],
                    start=(jc == 0),
                    stop=(jc == TCM - 1),
                )
            nc.vector.tensor_copy(ctxt[hr:hr + 64, hc, :], psc[:HS, :])
            stmp = stpool.tile([1, 512], F32, tag="stmp")
            nc.vector.tensor_copy(stmp[:], psc[HS:HS + 1, :])
            rf = stpool.tile([1, 512], F32, tag="rf")
            nc.vector.reciprocal_approx_fast(out=rf[:], in_=stmp[:])
            row = srf_ap[m * NH + h:m * NH + h + 1, :]
            nc.sync.dma_start(row, rf[0:1, :])
            bc = bcpool.tile([128, 512], F32, tag="bc")
            nc.sync.dma_start(bc[hr:hr + 64, :], row.to_broadcast((64, 512)))
            pending.append((h, bc))
            while len(pending) > 1:
                normalize_one()
        for f in fillers:
            f()
        del fillers[:]
        while pending:
            normalize_one()

    # Modality 0 bootstrap: kc-outer paired projection consumes x/W DMA
    # chunks as they arrive instead of waiting for whole tensors.
    emit_load_x(0)
    for which in ("q", "k"):
        st0 = mod[0]
        key = "qt" if which == "q" else "kt"
        st0[key] = (qpool if which == "q" else kpool).tile(
            [128, HC, PM], F16, tag=which, name=f"{which}t0")
        w = w_tiles["wq" if which == "q" else "wk"]
        for jcp in range(3):
            psA = ps_big.tile([128, 512], F32, tag="ps_big")
            psB = ps_big.tile([128, 512], F32, tag="ps_big")
            for kc in range(HC):
                nc.tensor.matmul(
                    psA[:], w[:, kc, (2 * jcp) * 128:(2 * jcp + 1) * 128],
                    st0["xt"][:, kc, :], start=(kc == 0), stop=(kc == HC - 1))
                nc.tensor.matmul(
                    psB[:], w[:, kc, (2 * jcp + 1) * 128:(2 * jcp + 2) * 128],
                    st0["xt"][:, kc, :], start=(kc == 0), stop=(kc == HC - 1))
            nc.vector.tensor_copy(st0[key][:, 2 * jcp, :], psA[:])
            nc.vector.tensor_copy(st0[key][:, 2 * jcp + 1, :], psB[:])
    for ti in range(TCM):
        for nn in range(2):
            proj_v_group(0, ti, nn)

    attention(0, list(phase_ab_fillers(1)))
    out_proj(0)
    attention(1, list(phase_ab_fillers(2)))
    out_proj(1)
    attention(2, list(phase_ab_fillers(3)))
    attention(3, out_proj_fillers(2))
    out_proj(3)


_NC_CACHE = {}


def build_nc():
    if "nc" not in _NC_CACHE:
        nc = bacc.Bacc("TRN2", target_bir_lowering=False, debug=False, num_devices=B)
        with TileContext(nc) as tc:
            with ExitStack() as stack:
                _emit(tc, stack)
        nc.compile()
        _NC_CACHE["nc"] = nc
    return _NC_CACHE["nc"]


def prep_in_maps(hidden_states, Wq, Wk, Wv, Wo):
    hs = np.asarray(hidden_states, dtype=np.float32)
    ws = {n: np.ascontiguousarray(np.asarray(w, dtype=np.float32)).astype(np.float16)
          for n, w in (("wq", Wq), ("wk", Wk), ("wv", Wv), ("wo", Wo))}
    return [
        {"x": np.ascontiguousarray(hs[b].reshape(T, H).T).astype(np.float16), **ws}
        for b in range(B)
    ]


def postprocess_out(arr):
    # device output is feature-major [H, T]; -> [M, PM, H] f32
    return arr.reshape(H, M, PM).transpose(1, 2, 0).astype(np.float32)


def _numpy_fallback(x, Wq, bq, Wk, bk, Wv, bv, Wo, bo):
    Bb, Mm, Pp, Hh = x.shape
    xx = x.reshape(-1, Hh)
    q = (xx @ Wq + bq).reshape(Bb, Mm, Pp, NH, HS).transpose(0, 1, 3, 2, 4)
    k = (xx @ Wk + bk).reshape(Bb, Mm, Pp, NH, HS).transpose(0, 1, 3, 2, 4)
    v = (xx @ Wv + bv).reshape(Bb, Mm, Pp, NH, HS).transpose(0, 1, 3, 2, 4)
    s = np.einsum("bmnqh,bmnkh->bmnqk", q, k) / np.sqrt(HS)
    s = s - s.max(axis=-1, keepdims=True)
    e = np.exp(s)
    p = e / e.sum(axis=-1, keepdims=True)
    ctx = np.einsum("bmnqk,bmnkh->bmnqh", p, v)
    ctx = ctx.transpose(0, 1, 3, 2, 4).reshape(Bb, Mm, Pp, Hh)
    return (ctx @ Wo + bo).astype(np.float32)


def kernel(hidden_states, Wq, bq, Wk, bk, Wv, bv, Wo, bo):
    hs = np.asarray(hidden_states, dtype=np.float32)
    biases = [np.asarray(b, dtype=np.float32) for b in (bq, bk, bv, bo)]
    if any(np.any(b) for b in biases):
        return _numpy_fallback(hs, np.asarray(Wq, dtype=np.float32), biases[0],
                               np.asarray(Wk, dtype=np.float32), biases[1],
                               np.asarray(Wv, dtype=np.float32), biases[2],
                               np.asarray(Wo, dtype=np.float32), biases[3])

    in_maps = prep_in_maps(hs, Wq, Wk, Wv, Wo)
    # The device occasionally comes up wedged from a previous process
    # (NRT_EXEC_UNIT_UNRECOVERABLE); retry, then degrade to the (correct
    # but slow) numpy path rather than crash.
    last_exc = None
    for _ in range(3):
        try:
            nc = build_nc()
            res = bass_utils.run_bass_kernel_spmd(
                nc, in_maps, core_ids=list(range(B)))
            return np.stack(
                [postprocess_out(res.results[b]["out"]) for b in range(B)])
        except Exception as e:  # noqa: BLE001
            last_exc = e
            import time
            time.sleep(2)
    import warnings
    warnings.warn(f"TRN execution failed ({last_exc!r}); numpy fallback")
    return _numpy_fallback(hs, np.asarray(Wq, dtype=np.float32), biases[0],
                           np.asarray(Wk, dtype=np.float32), biases[1],
                           np.asarray(Wv, dtype=np.float32), biases[2],
                           np.asarray(Wo, dtype=np.float32), biases[3])


# revision 17
# speedup vs baseline: 1.0815x; 1.0013x over previous
"""TRN2 Bass kernel for nn_Attention_m_17815524344494.

Multi-head attention over [B=8, M=4, P=512, H=768], nh=12, hs=64.
Sharding: data-parallel over batch B -> one batch element per NeuronCore (8 cores).

Per-core dataflow (T = M*P = 2048 tokens; all matmul operands fp16 —
FWL-eligible so LDWEIGHTS hides; accumulation is fp32 in PSUM):
  1. xT [768,2048] fp16 (pre-transposed+cast on host) DMA'd feature-major
     per modality on the gpsimd queue; weights fp16 split across the two
     HW-DGE queues (wq/wo on sync, wk/wv on scalar) so the bootstrap is
     not serialized on one software-DGE queue.
  2. qT = Wq^T xT, kT = Wk^T xT (feature-major), v = x Wv (token-major,
     augmented with a ones column per head for free softmax sums)
  3. per (modality, head): scoresT = kT^T q (keys on partitions),
     eT = exp(scoresT/8) via ScalarE, ctxT_unnorm/sums = v_aug^T eT,
     1/sums via reciprocal_approx_fast straight out of PSUM, partition-
     broadcast to 64 rows through a DRAM bounce DMA, normalize in place
     on VectorE two heads behind the producer (keeps the DVE queue from
     backing up at modality boundaries)
  4. out = ctxT^T Wo (token-major), f16 out DMA, host casts back to f32

PE idle is filled by weaving independent work into each attention
phase: modality m's attention interleaves modality m+1's projections,
and the last modality's attention interleaves modality m-1's output
projection (which is why the ctx pool is double-buffered).

Biases are zeros per the problem spec; a numpy fallback handles the
(never exercised) nonzero-bias case.
"""

from contextlib import ExitStack

import numpy as np

import concourse.mybir as mybir
from concourse import bacc, bass_utils
from concourse.tile import TileContext

F32 = mybir.dt.float32
F16 = mybir.dt.float16
AF = mybir.ActivationFunctionType
ALU = mybir.AluOpType

B, M, PM, H = 8, 4, 512, 768
NH, HS = 12, 64
T = M * PM          # 2048 tokens per core
HC = H // 128       # 6 hidden chunks
TCM = PM // 128     # 4 token chunks per modality


def _emit(tc, ctx):
    nc = tc.nc

    x_ap = nc.dram_tensor("x", [H, T], F16, kind="ExternalInput").ap()
    wq_ap = nc.dram_tensor("wq", [H, H], F16, kind="ExternalInput").ap()
    wk_ap = nc.dram_tensor("wk", [H, H], F16, kind="ExternalInput").ap()
    wv_ap = nc.dram_tensor("wv", [H, H], F16, kind="ExternalInput").ap()
    wo_ap = nc.dram_tensor("wo", [H, H], F16, kind="ExternalInput").ap()
    # Output stays feature-major [H, T]; the host transposes. This lets
    # out-proj run with Wo chunks stationary and ctxT moving (512-col
    # matmuls that hide LDWEIGHTS, and cc-accumulation that can start
    # before the last heads are normalized).
    out_ap = nc.dram_tensor("out", [H, T], F16, kind="ExternalOutput").ap()
    srf_ap = nc.dram_tensor("srf", [M * NH, 512], F32, kind="Internal").ap()

    const = ctx.enter_context(tc.tile_pool(name="const", bufs=1))

    onescol = const.tile([128, NH * TCM], F16)
    with tc.tile_pool(name="stage", bufs=1) as stage:
        ones_stage = stage.tile([128, 64], F32)
        nc.gpsimd.memset(ones_stage[:], 1.0)
        nc.vector.tensor_copy(onescol[:], ones_stage[:, :NH * TCM])

    wpool = ctx.enter_context(tc.tile_pool(name="w", bufs=1))
    xtp = ctx.enter_context(tc.tile_pool(name="xt", bufs=2))
    qpool = ctx.enter_context(tc.tile_pool(name="q", bufs=2))
    kpool = ctx.enter_context(tc.tile_pool(name="k", bufs=2))
    vpool = ctx.enter_context(tc.tile_pool(name="v", bufs=2))
    epool = ctx.enter_context(tc.tile_pool(name="e", bufs=8))
    stpool = ctx.enter_context(tc.tile_pool(name="st", bufs=2))
    bcpool = ctx.enter_context(tc.tile_pool(name="bc", bufs=4))
    cpool = ctx.enter_context(tc.tile_pool(name="ctx", bufs=2))
    opool = ctx.enter_context(tc.tile_pool(name="o", bufs=2))
    ps_big = ctx.enter_context(tc.tile_pool(name="ps_big", bufs=2, space="PSUM"))
    ps_sc = ctx.enter_context(tc.tile_pool(name="ps_sc", bufs=2, space="PSUM"))
    ps_c = ctx.enter_context(tc.tile_pool(name="ps_c", bufs=2, space="PSUM"))

    w_tiles = {}
    mod = {}

    def emit_load_x(m):
        xt = xtp.tile([128, HC, PM], F16, tag="xt")
        xsrc = x_ap.rearrange("(hc p) t -> p hc t", p=128)
        if m == 0:
            # The scheduler hoists the whole first accumulation group's DMA
            # waits into one shared-counter threshold, so the first matmul
            # effectively waits for ALL of x+wq: balance those 12 loads
            # evenly across the two HW-DGE queues (precise semaphores).
            # wk/wv/wo ride gpsimd's software DGE, whose laggy completion
            # visibility only the later k/v projections can tolerate.
            srcs = {}
            for name, ap in (("wq", wq_ap), ("wk", wk_ap),
                             ("wv", wv_ap), ("wo", wo_ap)):
                w_tiles[name] = wpool.tile([128, HC, H], F16, tag=name, name=name)
                srcs[name] = ap.rearrange("(kc p) j -> p kc j", p=128)
            for hc in range(HC):
                xe, we = (nc.sync, nc.scalar) if hc % 2 == 0 else (nc.scalar, nc.sync)
                xe.dma_start(xt[:, hc, :], xsrc[:, hc, :PM])
                we.dma_start(w_tiles["wq"][:, hc, :], srcs["wq"][:, hc, :])
            for name in ("wk", "wv", "wo"):
                for kc in range(HC):
                    nc.gpsimd.dma_start(
                        w_tiles[name][:, kc, :], srcs[name][:, kc, :])
        else:
            for hc in range(HC):
                nc.gpsimd.dma_start(xt[:, hc, :], xsrc[:, hc, m * PM:(m + 1) * PM])
        mod[m] = {"xt": xt}

    def proj_qk_group(m, which, jc):
        st = mod[m]
        key = "qt" if which == "q" else "kt"
        if key not in st:
            pool = qpool if which == "q" else kpool
            st[key] = pool.tile([128, HC, PM], F16, tag=which, name=f"{which}t")
        w = w_tiles["wq" if which == "q" else "wk"]
        ps = ps_big.tile([128, 512], F32, tag="ps_big")
        for kc in range(HC):
            nc.tensor.matmul(
                ps[:],
                w[:, kc, jc * 128:(jc + 1) * 128],
                st["xt"][:, kc, :],
                start=(kc == 0),
                stop=(kc == HC - 1),
            )
        if jc % 2 == 0:
            nc.vector.tensor_copy(st[key][:, jc, :], ps[:])
        else:
            nc.scalar.activation(st[key][:, jc, :], ps[:], AF.Copy)

    def proj_v_group(m, ti, nn):
        st = mod[m]
        if "vt" not in st:
            st["vt"] = vpool.tile([128, TCM, NH, HS + 1], F16, tag="v", name="vt")
            nc.vector.tensor_copy(
                st["vt"][:, :, :, HS],
                onescol[:].rearrange("p (t h) -> p t h", t=TCM),
            )
        ps = ps_big.tile([128, 512], F32, tag="ps_big")
        for kc in range(HC):
            nc.tensor.matmul(
                ps[:, :384],
                st["xt"][:, kc, ti * 128:(ti + 1) * 128],
                w_tiles["wv"][:, kc, nn * 384:(nn + 1) * 384],
                start=(kc == 0),
                stop=(kc == HC - 1),
            )
        nc.scalar.activation(
            st["vt"][:, ti, nn * 6:(nn + 1) * 6, :HS],
            ps[:, :384].rearrange("p (h c) -> p h c", c=HS),
            AF.Copy,
        )

    def phase_ab_fillers(m):
        # v groups are interleaved early: their ScalarE evacuations queue
        # behind exp ops, so spreading them across the attention phase beats
        # a burst at the modality boundary.
        yield lambda: emit_load_x(m)
        order = []
        for jc in range(HC):
            order.append(("q", jc))
        for jc in range(HC):
            order.append(("k", jc))
        vlist = [(ti, nn) for ti in range(TCM) for nn in range(2)]
        merged = []
        for i, qk in enumerate(order):
            merged.append(qk)
            if i % 3 == 1 and vlist:
                merged.append(("v", vlist.pop(0)))
        merged.extend(("v", v) for v in vlist)
        for item in merged:
            if item[0] == "v":
                ti, nn = item[1]
                yield lambda ti=ti, nn=nn: proj_v_group(m, ti, nn)
            else:
                which, jc = item
                yield lambda which=which, jc=jc: proj_qk_group(m, which, jc)

    out_dst = out_ap.rearrange("(oc p) t -> p oc t", p=128)

    def out_proj_piece(m, oc, osbs):
        # outT[oc*128: , m*512: ] = sum_cc Wo[cc,oc]^T ctxT[cc] -- Wo chunk
        # stationary, ctxT moving (512 cols hides LDWEIGHTS). cc runs in
        # order, so the first 4 matmuls only need heads 0..7 normalized and
        # the piece overlaps the tail of the attention normalize chain.
        ctxt = mod[m]["ctxt"]
        if oc == 0:
            osbs[m] = opool.tile([128, HC, PM], F16, tag="o", name="osb")
        osb = osbs[m]
        ps = ps_big.tile([128, 512], F32, tag="ps_big")
        for cc in range(HC):
            nc.tensor.matmul(
                ps[:],
                w_tiles["wo"][:, cc, oc * 128:(oc + 1) * 128],
                ctxt[:, cc, :],
                start=(cc == 0),
                stop=(cc == HC - 1),
            )
        nc.scalar.activation(osb[:, oc, :], ps[:], AF.Copy)
        nc.sync.dma_start(
            out_dst[:, oc, m * PM:(m + 1) * PM], osb[:, oc, :])

    def out_proj_fillers(m):
        osbs = {}
        return [
            (lambda oc=oc: out_proj_piece(m, oc, osbs))
            for oc in range(HC)
        ]

    def out_proj(m):
        for f in out_proj_fillers(m):
            f()

    def attention(m, fillers):
        # Per (modality, head): scoresT on PE, exp on ScalarE, PV (with the
        # v_aug ones column producing softmax sums in psum row 64).
        # 1/sums comes straight off PSUM via reciprocal_approx_fast, is
        # partition-broadcast through a DRAM bounce DMA into the head's own
        # 64 rows, and the in-place normalize trails the producer by two
        # heads so the (in-order) DVE queue never gates the PE. Between each
        # head's scores and PV one filler runs -- independent PE work that
        # fills the exp wait.
        st = mod[m]
        qt, kt, vt = st["qt"], st["kt"], st["vt"]
        ctxt = cpool.tile([128, HC, PM], F16, tag="ctx")
        st["ctxt"] = ctxt
        pending = []

        def normalize_one():
            h, bc = pending.pop(0)
            hc, hr = h // 2, (h % 2) * 64
            nc.vector.tensor_tensor(
                ctxt[hr:hr + 64, hc, :], ctxt[hr:hr + 64, hc, :],
                bc[hr:hr + 64, :], ALU.mult,
            )

        for h in range(NH):
            hc, hr = h // 2, (h % 2) * 64
            qh = qt[hr:hr + 64, hc, :]
            # Scores land pairwise in a 2-bank PSUM tile so ONE [128,1024]
            # exp evacuates both key-chunks (fewer ScalarE ops, less
            # fixed-cost per element).
            ets = []
            for jp in range(TCM // 2):
                pssc = ps_sc.tile([128, 2, 512], F32, tag="ps_sc")
                for half in range(2):
                    jc = 2 * jp + half
                    nc.tensor.matmul(
                        pssc[:, half, :],
                        kt[hr:hr + 64, hc, jc * 128:(jc + 1) * 128],
                        qh,
                        start=True,
                        stop=True,
                    )
                et = epool.tile([128, 2, 512], F16, tag="e")
                nc.scalar.activation(et[:], pssc[:], AF.Exp, scale=0.125)
                ets.append(et)
            if fillers:
                fillers.pop(0)()
            psc = ps_c.tile([HS + 1, 512], F32, tag="ps_c")
            for jc in range(TCM):
                nc.tensor.matmul(
                    psc[:],
                    vt[:, jc, h, :],
                    ets[jc // 2][:, jc % 2, :],
                    start=(jc == 0),
                    stop=(jc == TCM - 1),
                )
            nc.vector.tensor_copy(ctxt[hr:hr + 64, hc, :], psc[:HS, :])
            stmp = stpool.tile([1, 512], F32, tag="stmp")
            nc.vector.tensor_copy(stmp[:], psc[HS:HS + 1, :])
            rf = stpool.tile([1, 512], F32, tag="rf")
            nc.vector.reciprocal_approx_fast(out=rf[:], in_=stmp[:])
            row = srf_ap[m * NH + h:m * NH + h + 1, :]
            nc.sync.dma_start(row, rf[0:1, :])
            bc = bcpool.tile([128, 512], F32, tag="bc")
            nc.sync.dma_start(bc[hr:hr + 64, :], row.to_broadcast((64, 512)))
            pending.append((h, bc))
            while len(pending) > 1:
                normalize_one()
        for f in fillers:
            f()
        del fillers[:]
        while pending:
            normalize_one()

    # Modality 0 bootstrap: kc-outer paired projection consumes x/W DMA
    # chunks as they arrive instead of waiting for whole tensors.
    emit_load_x(0)
    for which in ("q", "k"):
        st0 = mod[0]
        key = "qt" if which == "q" else "kt"
        st0[key] = (qpool if which == "q" else kpool).tile(
            [128, HC, PM], F16, tag=which, name=f"{which}t0")
        w = w_tiles["wq" if which == "q" else "wk"]
        for jcp in range(3):
            psA = ps_big.tile([128, 512], F32, tag="ps_big")
            psB = ps_big.tile([128, 512], F32, tag="ps_big")
            for kc in range(HC):
                nc.tensor.matmul(
                    psA[:], w[:, kc, (2 * jcp) * 128:(2 * jcp + 1) * 128],
                    st0["xt"][:, kc, :], start=(kc == 0), stop=(kc == HC - 1))
                nc.tensor.matmul(
                    psB[:], w[:, kc, (2 * jcp + 1) * 128:(2 * jcp + 2) * 128],
                    st0["xt"][:, kc, :], start=(kc == 0), stop=(kc == HC - 1))
            nc.vector.tensor_copy(st0[key][:, 2 * jcp, :], psA[:])
            nc.vector.tensor_copy(st0[key][:, 2 * jcp + 1, :], psB[:])
    for ti in range(TCM):
        for nn in range(2):
            proj_v_group(0, ti, nn)

    attention(0, list(phase_ab_fillers(1)))
    out_proj(0)
    attention(1, list(phase_ab_fillers(2)))
    out_proj(1)
    attention(2, list(phase_ab_fillers(3)))
    attention(3, out_proj_fillers(2))
    out_proj(3)


_NC_CACHE = {}


def build_nc():
    if "nc" not in _NC_CACHE:
        nc = bacc.Bacc("TRN2", target_bir_lowering=False, debug=False, num_devices=B)
        with TileContext(nc) as tc:
            with ExitStack() as stack:
                _emit(tc, stack)
        nc.compile()
        _NC_CACHE["nc"] = nc
    return _NC_CACHE["nc"]


def prep_in_maps(hidden_states, Wq, Wk, Wv, Wo):
    hs = np.asarray(hidden_states, dtype=np.float32)
    ws = {n: np.ascontiguousarray(np.asarray(w, dtype=np.float32)).astype(np.float16)
          for n, w in (("wq", Wq), ("wk", Wk), ("wv", Wv), ("wo", Wo))}
    return [
        {"x": np.ascontiguousarray(hs[b].reshape(T, H).T).astype(np.float16), **ws}
        for b in range(B)
    ]


def postprocess_out(arr):
    # device output is feature-major [H, T]; -> [M, PM, H] f32
    return arr.reshape(H, M, PM).transpose(1, 2, 0).astype(np.float32)


def _numpy_fallback(x, Wq, bq, Wk, bk, Wv, bv, Wo, bo):
    Bb, Mm, Pp, Hh = x.shape
    xx = x.reshape(-1, Hh)
    q = (xx @ Wq + bq).reshape(Bb, Mm, Pp, NH, HS).transpose(0, 1, 3, 2, 4)
    k = (xx @ Wk + bk).reshape(Bb, Mm, Pp, NH, HS).transpose(0, 1, 3, 2, 4)
    v = (xx @ Wv + bv).reshape(Bb, Mm, Pp, NH, HS).transpose(0, 1, 3, 2, 4)
    s = np.einsum("bmnqh,bmnkh->bmnqk", q, k) / np.sqrt(HS)
    s = s - s.max(axis=-1, keepdims=True)
    e = np.exp(s)
    p = e / e.sum(axis=-1, keepdims=True)
    ctx = np.einsum("bmnqk,bmnkh->bmnqh", p, v)
    ctx = ctx.transpose(0, 1, 3, 2, 4).reshape(Bb, Mm, Pp, Hh)
    return (ctx @ Wo + bo).astype(np.float32)


def kernel(hidden_states, Wq, bq, Wk, bk, Wv, bv, Wo, bo):
    hs = np.asarray(hidden_states, dtype=np.float32)
    biases = [np.asarray(b, dtype=np.float32) for b in (bq, bk, bv, bo)]
    if any(np.any(b) for b in biases):
        return _numpy_fallback(hs, np.asarray(Wq, dtype=np.float32), biases[0],
                               np.asarray(Wk, dtype=np.float32), biases[1],
                               np.asarray(Wv, dtype=np.float32), biases[2],
                               np.asarray(Wo, dtype=np.float32), biases[3])

    in_maps = prep_in_maps(hs, Wq, Wk, Wv, Wo)
    # The device occasionally comes up wedged from a previous process
    # (NRT_EXEC_UNIT_UNRECOVERABLE); retry, then degrade to the (correct
    # but slow) numpy path rather than crash.
    last_exc = None
    for _ in range(3):
        try:
            nc = build_nc()
            res = bass_utils.run_bass_kernel_spmd(
                nc, in_maps, core_ids=list(range(B)))
            return np.stack(
                [postprocess_out(res.results[b]["out"]) for b in range(B)])
        except Exception as e:  # noqa: BLE001
            last_exc = e
            import time
            time.sleep(2)
    import warnings
    warnings.warn(f"TRN execution failed ({last_exc!r}); numpy fallback")
    return _numpy_fallback(hs, np.asarray(Wq, dtype=np.float32), biases[0],
                           np.asarray(Wk, dtype=np.float32), biases[1],
                           np.asarray(Wv, dtype=np.float32), biases[2],
                           np.asarray(Wo, dtype=np.float32), biases[3])


# revision 19
# speedup vs baseline: 1.0954x; 1.0128x over previous
"""TRN2 Bass kernel for nn_Attention_m_17815524344494.

Multi-head attention over [B=8, M=4, P=512, H=768], nh=12, hs=64.
Sharding: data-parallel over batch B -> one batch element per NeuronCore (8 cores).

Per-core dataflow (T = M*P = 2048 tokens; all matmul operands fp16 —
FWL-eligible so LDWEIGHTS hides; accumulation is fp32 in PSUM):
  1. xT [768,2048] fp16 (pre-transposed+cast on host) DMA'd feature-major
     per modality on the gpsimd queue; weights fp16 split across the two
     HW-DGE queues (wq/wo on sync, wk/wv on scalar) so the bootstrap is
     not serialized on one software-DGE queue.
  2. qT = Wq^T xT, kT = Wk^T xT (feature-major), v = x Wv (token-major,
     augmented with a ones column per head for free softmax sums)
  3. per (modality, head): scoresT = kT^T q (keys on partitions),
     eT = exp(scoresT/8) via ScalarE, ctxT_unnorm/sums = v_aug^T eT,
     1/sums via reciprocal_approx_fast straight out of PSUM, partition-
     broadcast to 64 rows through a DRAM bounce DMA, normalize in place
     on VectorE two heads behind the producer (keeps the DVE queue from
     backing up at modality boundaries)
  4. out = ctxT^T Wo (token-major), f16 out DMA, host casts back to f32

PE idle is filled by weaving independent work into each attention
phase: modality m's attention interleaves modality m+1's projections,
and the last modality's attention interleaves modality m-1's output
projection (which is why the ctx pool is double-buffered).

Biases are zeros per the problem spec; a numpy fallback handles the
(never exercised) nonzero-bias case.
"""

from contextlib import ExitStack

import numpy as np

import concourse.mybir as mybir
from concourse import bacc, bass_utils
from concourse.tile import TileContext

F32 = mybir.dt.float32
F16 = mybir.dt.float16
AF = mybir.ActivationFunctionType
ALU = mybir.AluOpType

B, M, PM, H = 8, 4, 512, 768
NH, HS = 12, 64
T = M * PM          # 2048 tokens per core
HC = H // 128       # 6 hidden chunks
TCM = PM // 128     # 4 token chunks per modality


def _emit(tc, ctx):
    nc = tc.nc

    x_ap = nc.dram_tensor("x", [H, T], F16, kind="ExternalInput").ap()
    wq_ap = nc.dram_tensor("wq", [H, H], F16, kind="ExternalInput").ap()
    wk_ap = nc.dram_tensor("wk", [H, H], F16, kind="ExternalInput").ap()
    wv_ap = nc.dram_tensor("wv", [H, H], F16, kind="ExternalInput").ap()
    wo_ap = nc.dram_tensor("wo", [H, H], F16, kind="ExternalInput").ap()
    # Output stays feature-major [H, T]; the host transposes. This lets
    # out-proj run with Wo chunks stationary and ctxT moving (512-col
    # matmuls that hide LDWEIGHTS, and cc-accumulation that can start
    # before the last heads are normalized).
    out_ap = nc.dram_tensor("out", [H, T], F16, kind="ExternalOutput").ap()
    srf_ap = nc.dram_tensor("srf", [M * NH, 512], F32, kind="Internal").ap()

    const = ctx.enter_context(tc.tile_pool(name="const", bufs=1))

    onescol = const.tile([128, NH * TCM], F16)
    with tc.tile_pool(name="stage", bufs=1) as stage:
        ones_stage = stage.tile([128, 64], F32)
        nc.gpsimd.memset(ones_stage[:], 1.0)
        nc.vector.tensor_copy(onescol[:], ones_stage[:, :NH * TCM])

    wpool = ctx.enter_context(tc.tile_pool(name="w", bufs=1))
    xtp = ctx.enter_context(tc.tile_pool(name="xt", bufs=2))
    qpool = ctx.enter_context(tc.tile_pool(name="q", bufs=2))
    kpool = ctx.enter_context(tc.tile_pool(name="k", bufs=2))
    vpool = ctx.enter_context(tc.tile_pool(name="v", bufs=2))
    epool = ctx.enter_context(tc.tile_pool(name="e", bufs=8))
    stpool = ctx.enter_context(tc.tile_pool(name="st", bufs=2))
    bcpool = ctx.enter_context(tc.tile_pool(name="bc", bufs=4))
    cpool = ctx.enter_context(tc.tile_pool(name="ctx", bufs=2))
    opool = ctx.enter_context(tc.tile_pool(name="o", bufs=2))
    ps_big = ctx.enter_context(tc.tile_pool(name="ps_big", bufs=2, space="PSUM"))
    ps_sc = ctx.enter_context(tc.tile_pool(name="ps_sc", bufs=2, space="PSUM"))
    ps_c = ctx.enter_context(tc.tile_pool(name="ps_c", bufs=2, space="PSUM"))

    w_tiles = {}
    mod = {}

    def emit_load_x(m):
        xt = xtp.tile([128, HC, PM], F16, tag="xt")
        xsrc = x_ap.rearrange("(hc p) t -> p hc t", p=128)
        if m == 0:
            # The scheduler hoists the whole first accumulation group's DMA
            # waits into one shared-counter threshold, so the first matmul
            # effectively waits for ALL of x+wq: balance those 12 loads
            # evenly across the two HW-DGE queues (precise semaphores).
            # wk/wv/wo ride gpsimd's software DGE, whose laggy completion
            # visibility only the later k/v projections can tolerate.
            srcs = {}
            for name, ap in (("wq", wq_ap), ("wk", wk_ap),
                             ("wv", wv_ap), ("wo", wo_ap)):
                w_tiles[name] = wpool.tile([128, HC, H], F16, tag=name, name=name)
                srcs[name] = ap.rearrange("(kc p) j -> p kc j", p=128)
            # Pairwise interleave so (x[kc], wq[kc]) complete together: the
            # kc-major bootstrap consumes pairs in arrival order. 12 HWDGE
            # loads stay near the 8 wrap-free semaphore lanes; wk/wv/wo ride
            # the independent software-DGE lanes.
            for hc in range(HC):
                xe, we = (nc.sync, nc.scalar) if hc % 2 == 0 else (nc.scalar, nc.sync)
                xe.dma_start(xt[:, hc, :], xsrc[:, hc, :PM])
                we.dma_start(w_tiles["wq"][:, hc, :], srcs["wq"][:, hc, :])
            for name in ("wk", "wv", "wo"):
                for kc in range(HC):
                    nc.gpsimd.dma_start(
                        w_tiles[name][:, kc, :], srcs[name][:, kc, :])
        else:
            for hc in range(HC):
                nc.gpsimd.dma_start(xt[:, hc, :], xsrc[:, hc, m * PM:(m + 1) * PM])
        mod[m] = {"xt": xt}

    def proj_qk_group(m, which, jc):
        st = mod[m]
        key = "qt" if which == "q" else "kt"
        if key not in st:
            pool = qpool if which == "q" else kpool
            st[key] = pool.tile([128, HC, PM], F16, tag=which, name=f"{which}t")
        w = w_tiles["wq" if which == "q" else "wk"]
        ps = ps_big.tile([128, 512], F32, tag="ps_big")
        for kc in range(HC):
            nc.tensor.matmul(
                ps[:],
                w[:, kc, jc * 128:(jc + 1) * 128],
                st["xt"][:, kc, :],
                start=(kc == 0),
                stop=(kc == HC - 1),
            )
        if jc % 2 == 0:
            nc.vector.tensor_copy(st[key][:, jc, :], ps[:])
        else:
            nc.scalar.activation(st[key][:, jc, :], ps[:], AF.Copy)

    def proj_v_group(m, ti, nn):
        st = mod[m]
        if "vt" not in st:
            st["vt"] = vpool.tile([128, TCM, NH, HS + 1], F16, tag="v", name="vt")
            nc.vector.tensor_copy(
                st["vt"][:, :, :, HS],
                onescol[:].rearrange("p (t h) -> p t h", t=TCM),
            )
        ps = ps_big.tile([128, 512], F32, tag="ps_big")
        for kc in range(HC):
            nc.tensor.matmul(
                ps[:, :384],
                st["xt"][:, kc, ti * 128:(ti + 1) * 128],
                w_tiles["wv"][:, kc, nn * 384:(nn + 1) * 384],
                start=(kc == 0),
                stop=(kc == HC - 1),
            )
        nc.scalar.activation(
            st["vt"][:, ti, nn * 6:(nn + 1) * 6, :HS],
            ps[:, :384].rearrange("p (h c) -> p h c", c=HS),
            AF.Copy,
        )

    def phase_ab_fillers(m):
        # v groups are interleaved early: their ScalarE evacuations queue
        # behind exp ops, so spreading them across the attention phase beats
        # a burst at the modality boundary.
        yield lambda: emit_load_x(m)
        order = []
        for jc in range(HC):
            order.append(("q", jc))
        for jc in range(HC):
            order.append(("k", jc))
        vlist = [(ti, nn) for ti in range(TCM) for nn in range(2)]
        merged = []
        for i, qk in enumerate(order):
            merged.append(qk)
            if i % 3 == 1 and vlist:
                merged.append(("v", vlist.pop(0)))
        merged.extend(("v", v) for v in vlist)
        for item in merged:
            if item[0] == "v":
                ti, nn = item[1]
                yield lambda ti=ti, nn=nn: proj_v_group(m, ti, nn)
            else:
                which, jc = item
                yield lambda which=which, jc=jc: proj_qk_group(m, which, jc)

    out_dst = out_ap.rearrange("(oc p) t -> p oc t", p=128)

    def out_proj_piece(m, oc, osbs):
        # outT[oc*128: , m*512: ] = sum_cc Wo[cc,oc]^T ctxT[cc] -- Wo chunk
        # stationary, ctxT moving (512 cols hides LDWEIGHTS). cc runs in
        # order, so the first 4 matmuls only need heads 0..7 normalized and
        # the piece overlaps the tail of the attention normalize chain.
        ctxt = mod[m]["ctxt"]
        if oc == 0:
            osbs[m] = opool.tile([128, HC, PM], F16, tag="o", name="osb")
        osb = osbs[m]
        ps = ps_big.tile([128, 512], F32, tag="ps_big")
        for cc in range(HC):
            nc.tensor.matmul(
                ps[:],
                w_tiles["wo"][:, cc, oc * 128:(oc + 1) * 128],
                ctxt[:, cc, :],
                start=(cc == 0),
                stop=(cc == HC - 1),
            )
        nc.scalar.activation(osb[:, oc, :], ps[:], AF.Copy)
        nc.sync.dma_start(
            out_dst[:, oc, m * PM:(m + 1) * PM], osb[:, oc, :])

    def out_proj_fillers(m):
        osbs = {}
        return [
            (lambda oc=oc: out_proj_piece(m, oc, osbs))
            for oc in range(HC)
        ]

    def out_proj(m):
        for f in out_proj_fillers(m):
            f()

    def attention(m, fillers):
        # Per (modality, head): scoresT on PE, exp on ScalarE, PV (with the
        # v_aug ones column producing softmax sums in psum row 64).
        # 1/sums comes straight off PSUM via reciprocal_approx_fast, is
        # partition-broadcast through a DRAM bounce DMA into the head's own
        # 64 rows, and the in-place normalize trails the producer by two
        # heads so the (in-order) DVE queue never gates the PE. Between each
        # head's scores and PV one filler runs -- independent PE work that
        # fills the exp wait.
        st = mod[m]
        qt, kt, vt = st["qt"], st["kt"], st["vt"]
        ctxt = cpool.tile([128, HC, PM], F16, tag="ctx")
        st["ctxt"] = ctxt
        pending = []

        def normalize_one():
            h, bc = pending.pop(0)
            hc, hr = h // 2, (h % 2) * 64
            nc.vector.tensor_tensor(
                ctxt[hr:hr + 64, hc, :], ctxt[hr:hr + 64, hc, :],
                bc[hr:hr + 64, :], ALU.mult,
            )

        for h in range(NH):
            hc, hr = h // 2, (h % 2) * 64
            qh = qt[hr:hr + 64, hc, :]
            # Scores land pairwise in a 2-bank PSUM tile so ONE [128,1024]
            # exp evacuates both key-chunks (fewer ScalarE ops, less
            # fixed-cost per element).
            ets = []
            for jp in range(TCM // 2):
                pssc = ps_sc.tile([128, 2, 512], F32, tag="ps_sc")
                for half in range(2):
                    jc = 2 * jp + half
                    nc.tensor.matmul(
                        pssc[:, half, :],
                        kt[hr:hr + 64, hc, jc * 128:(jc + 1) * 128],
                        qh,
                        start=True,
                        stop=True,
                    )
                et = epool.tile([128, 2, 512], F16, tag="e")
                nc.scalar.activation(et[:], pssc[:], AF.Exp, scale=0.125)
                ets.append(et)
            if fillers:
                fillers.pop(0)()
            psc = ps_c.tile([HS + 1, 512], F32, tag="ps_c")
            for jc in range(TCM):
                nc.tensor.matmul(
                    psc[:],
                    vt[:, jc, h, :],
                    ets[jc // 2][:, jc % 2, :],
                    start=(jc == 0),
                    stop=(jc == TCM - 1),
                )
            nc.vector.tensor_copy(ctxt[hr:hr + 64, hc, :], psc[:HS, :])
            stmp = stpool.tile([1, 512], F32, tag="stmp")
            nc.vector.tensor_copy(stmp[:], psc[HS:HS + 1, :])
            rf = stpool.tile([1, 512], F32, tag="rf")
            nc.vector.reciprocal_approx_fast(out=rf[:], in_=stmp[:])
            row = srf_ap[m * NH + h:m * NH + h + 1, :]
            nc.sync.dma_start(row, rf[0:1, :])
            bc = bcpool.tile([128, 512], F32, tag="bc")
            nc.sync.dma_start(bc[hr:hr + 64, :], row.to_broadcast((64, 512)))
            pending.append((h, bc))
            while len(pending) > 1:
                normalize_one()
        for f in fillers:
            f()
        del fillers[:]
        while pending:
            normalize_one()

    # Modality 0 bootstrap. The q projection runs kc-major with all six
    # jc accumulation groups open at once (6 of the 8 PSUM banks), so each
    # (x[kc], wq[kc]) chunk pair is consumed the moment it lands -- the PE
    # starts on the first pair instead of waiting for the full tensors.
    emit_load_x(0)
    st0 = mod[0]
    st0["qt"] = qpool.tile([128, HC, PM], F16, tag="q", name="qt0")
    gA = ps_big.tile([128, 512], F32, tag="ps_big", name="gA")
    gB = ps_big.tile([128, 512], F32, tag="ps_big", name="gB")
    gCD = ps_sc.tile([128, 2, 512], F32, tag="ps_sc", name="gCD")
    gE = ps_c.tile([128, 512], F32, tag="ps_c", name="gE")
    gF = ps_c.tile([128, 512], F32, tag="ps_c", name="gF")
    groups = [gA[:], gB[:], gCD[:, 0, :], gCD[:, 1, :], gE[:], gF[:]]
    wq_t = w_tiles["wq"]
    for kc in range(HC):
        for jc in range(HC):
            nc.tensor.matmul(
                groups[jc], wq_t[:, kc, jc * 128:(jc + 1) * 128],
                st0["xt"][:, kc, :], start=(kc == 0), stop=(kc == HC - 1))
    for jc in range(HC):
        if jc % 2 == 0:
            nc.vector.tensor_copy(st0["qt"][:, jc, :], groups[jc])
        else:
            nc.scalar.activation(st0["qt"][:, jc, :], groups[jc], AF.Copy)
    for jc in range(HC):
        proj_qk_group(0, "k", jc)
    for ti in range(TCM):
        for nn in range(2):
            proj_v_group(0, ti, nn)

    attention(0, list(phase_ab_fillers(1)))
    out_proj(0)
    attention(1, list(phase_ab_fillers(2)))
    out_proj(1)
    attention(2, list(phase_ab_fillers(3)))
    attention(3, out_proj_fillers(2))
    out_proj(3)


_NC_CACHE = {}


def build_nc():
    if "nc" not in _NC_CACHE:
        nc = bacc.Bacc("TRN2", target_bir_lowering=False, debug=False, num_devices=B)
        with TileContext(nc) as tc:
            with ExitStack() as stack:
                _emit(tc, stack)
        nc.compile()
        _NC_CACHE["nc"] = nc
    return _NC_CACHE["nc"]


def prep_in_maps(hidden_states, Wq, Wk, Wv, Wo):
    hs = np.asarray(hidden_states, dtype=np.float32)
    ws = {n: np.ascontiguousarray(np.asarray(w, dtype=np.float32)).astype(np.float16)
          for n, w in (("wq", Wq), ("wk", Wk), ("wv", Wv), ("wo", Wo))}
    return [
        {"x": np.ascontiguousarray(hs[b].reshape(T, H).T).astype(np.float16), **ws}
        for b in range(B)
    ]


def postprocess_out(arr):
    # device output is feature-major [H, T]; -> [M, PM, H] f32
    return arr.reshape(H, M, PM).transpose(1, 2, 0).astype(np.float32)


def _numpy_fallback(x, Wq, bq, Wk, bk, Wv, bv, Wo, bo):
    Bb, Mm, Pp, Hh = x.shape
    xx = x.reshape(-1, Hh)
    q = (xx @ Wq + bq).reshape(Bb, Mm, Pp, NH, HS).transpose(0, 1, 3, 2, 4)
    k = (xx @ Wk + bk).reshape(Bb, Mm, Pp, NH, HS).transpose(0, 1, 3, 2, 4)
    v = (xx @ Wv + bv).reshape(Bb, Mm, Pp, NH, HS).transpose(0, 1, 3, 2, 4)
    s = np.einsum("bmnqh,bmnkh->bmnqk", q, k) / np.sqrt(HS)
    s = s - s.max(axis=-1, keepdims=True)
    e = np.exp(s)
    p = e / e.sum(axis=-1, keepdims=True)
    ctx = np.einsum("bmnqk,bmnkh->bmnqh", p, v)
    ctx = ctx.transpose(0, 1, 3, 2, 4).reshape(Bb, Mm, Pp, Hh)
    return (ctx @ Wo + bo).astype(np.float32)


def kernel(hidden_states, Wq, bq, Wk, bk, Wv, bv, Wo, bo):
    hs = np.asarray(hidden_states, dtype=np.float32)
    biases = [np.asarray(b, dtype=np.float32) for b in (bq, bk, bv, bo)]
    if any(np.any(b) for b in biases):
        return _numpy_fallback(hs, np.asarray(Wq, dtype=np.float32), biases[0],
                               np.asarray(Wk, dtype=np.float32), biases[1],
                               np.asarray(Wv, dtype=np.float32), biases[2],
                               np.asarray(Wo, dtype=np.float32), biases[3])

    in_maps = prep_in_maps(hs, Wq, Wk, Wv, Wo)
    # The device occasionally comes up wedged from a previous process
    # (NRT_EXEC_UNIT_UNRECOVERABLE); retry, then degrade to the (correct
    # but slow) numpy path rather than crash.
    last_exc = None
    for _ in range(3):
        try:
            nc = build_nc()
            res = bass_utils.run_bass_kernel_spmd(
                nc, in_maps, core_ids=list(range(B)))
            return np.stack(
                [postprocess_out(res.results[b]["out"]) for b in range(B)])
        except Exception as e:  # noqa: BLE001
            last_exc = e
            import time
            time.sleep(2)
    import warnings
    warnings.warn(f"TRN execution failed ({last_exc!r}); numpy fallback")
    return _numpy_fallback(hs, np.asarray(Wq, dtype=np.float32), biases[0],
                           np.asarray(Wk, dtype=np.float32), biases[1],
                           np.asarray(Wv, dtype=np.float32), biases[2],
                           np.asarray(Wo, dtype=np.float32), biases[3])
